# revision 23
# baseline (speedup 1.0000x reference)
"""2-layer RNN on 8 TRN2 cores — layer-pipelined across core pairs.

Structure: 4 batch groups x 2 pipeline stages. Pair p = (core 2p, core
2p+1) handles batch group p (BL=16 sequences):
  - even core ("stage0"): layer-0 input GEMM (on x) + layer-0 recurrence;
    ships out0 chunks to its partner via pair AllGather.
  - odd core ("stage1"): layer-1 input GEMM (on received out0) + layer-1
    recurrence + FC head.

vs. the data-parallel baseline (each core: both layers, batch 8), each
weight load in the LDWEIGHTS-bound recurrence now serves 16 batch
elements instead of 8, halving per-core scan steps (512 vs 1024).

Per-core divergence uses tc.If(partition_id % 2) branches; the pair
collectives sit outside all control flow, with stage1's send skipped
(its internal send buffer ships garbage that nobody reads).
"""

import numpy as np
import ml_dtypes

B, S, I, H, C = 64, 512, 256, 512, 10
NCORES = 8
BL = B // (NCORES // 2)  # 16 sequences per pair
CH = 32                  # timesteps per shipped chunk
NCH = S // CH
NSLOT = 4                # recv ring slots

_cache = {}


def _build_nc():
    import concourse.bass as bass
    import concourse.bacc as bacc
    import concourse.tile as tile
    from concourse.bass import mybir

    import bass_rust

    f32 = mybir.dt.float32
    bf16 = mybir.dt.bfloat16
    Tanh = mybir.ActivationFunctionType.Tanh
    PAIRS = [[0, 1], [2, 3], [4, 5], [6, 7]]
    add_dep = bass_rust.add_dep_helper

    nc = bacc.Bacc("TRN2", target_bir_lowering=False, debug=False, num_devices=NCORES)

    xT_d = nc.dram_tensor("xT", [128, 2, S * BL], bf16, kind="ExternalInput")
    wL_d = nc.dram_tensor("wL", [128, 2, 4, 128], bf16, kind="ExternalInput")
    wR_d = nc.dram_tensor("wR", [128, 4, 4, 128], bf16, kind="ExternalInput")
    whh_d = nc.dram_tensor("whh", [128, 4, 4, 128], bf16, kind="ExternalInput")
    bias_d = nc.dram_tensor("bias", [128, 4], f32, kind="ExternalInput")
    wfc_d = nc.dram_tensor("wfc", [128, 4, C], bf16, kind="ExternalInput")
    bfc_d = nc.dram_tensor("bfc", [C, 1], f32, kind="ExternalInput")
    id_d = nc.dram_tensor("ident", [128, 128], bf16, kind="ExternalInput")
    out_d = nc.dram_tensor("out", [C, BL], f32, kind="ExternalOutput")

    with tile.TileContext(nc) as tc:
        with tc.tile_pool(name="sb", bufs=1) as sb, tc.tile_pool(
            name="ps", bufs=1, space="PSUM"
        ) as psp, tc.tile_pool(name="dr", bufs=1, space="DRAM") as dr:
            send_d = [
                dr.tile([128, CH, 4, BL], bf16, name=f"send{k}") for k in range(NCH)
            ]
            gath_d = [
                dr.tile([256, CH, 4, BL], bf16, name=f"gath{k}") for k in range(NCH)
            ]
            xT = sb.tile([128, 2, S * BL], bf16)
            pre = sb.tile([128, S, 4, BL], bf16)
            seq = sb.tile([128, S, 4, BL], bf16)
            recv = sb.tile([128, NSLOT, CH, 4, BL], bf16)
            wL = sb.tile([128, 2, 4, 128], bf16)
            wR = sb.tile([128, 4, 4, 128], bf16)
            whh = sb.tile([128, 4, 4, 128], bf16)
            bias = sb.tile([128, 4], f32)
            wfc = sb.tile([128, 4, C], bf16)
            bfc = sb.tile([C, 1], f32)
            ident = sb.tile([128, 128], bf16)
            fco = sb.tile([C, BL], f32)

            pid = nc.partition_id()
            probe = sb.tile([1, 1, 1, 1], bf16)

            # shared prologue: recurrence weights everyone needs
            nc.sync.dma_start(whh[:], whh_d[:])
            nc.sync.dma_start(bias[:], bias_d[:])
            nc.sync.dma_start(ident[:], id_d[:])
            # stage-specific loads
            with tc.If(pid % 2 == 0) as c0:
                nc.sync.dma_start(wL[:], wL_d[:])
                nc.sync.dma_start(xT[:, :, 0 : 2 * CH * BL], xT_d[:, :, 0 : 2 * CH * BL])
                nc.sync.dma_start(
                    xT[:, :, 2 * CH * BL : 8 * CH * BL],
                    xT_d[:, :, 2 * CH * BL : 8 * CH * BL],
                )
                nc.sync.dma_start(xT[:, :, 8 * CH * BL :], xT_d[:, :, 8 * CH * BL :])
            with c0.Else():
                nc.sync.dma_start(wR[:], wR_d[:])
                nc.sync.dma_start(wfc[:], wfc_d[:])
                nc.sync.dma_start(bfc[:], bfc_d[:])

            gps = [psp.tile([128, CH, BL], f32, name=f"gps{i}") for i in range(4)]
            # Recurrence PSUM: one small [4jc, BL] tile per step, rotating
            # over 4 tiles so the tanh of step t-1 (and its WAR hazard)
            # never gates step t's matmuls; full 16-batch free dim per MM.
            sps = [psp.tile([128, 4, BL], f32, name=f"sps{i}") for i in range(4)]

            def gemm_L(k):
                """Stage0 input GEMM for chunk k (from x)."""
                t0 = k * CH
                for jc in range(4):
                    ps = gps[jc]
                    for kc in range(2):
                        nc.tensor.matmul(
                            ps[:],
                            wL[:, kc, jc, :],
                            xT[:, kc, t0 * BL : (t0 + CH) * BL],
                            start=(kc == 0),
                            stop=(kc == 1),
                        )
                    nc.vector.tensor_scalar_add(
                        pre[:, t0 : t0 + CH, jc, :], ps[:], bias[:, jc : jc + 1]
                    )

            def gemm_R(k):
                """Stage1 input GEMM for chunk k (from received out0)."""
                t0 = k * CH
                sl = k % NSLOT
                for jc in range(4):
                    ps = gps[jc]
                    for kc in range(4):
                        nc.tensor.matmul(
                            ps[:],
                            wR[:, kc, jc, :],
                            recv[:, sl, :, kc, :],
                            start=(kc == 0),
                            stop=(kc == 3),
                        )
                    nc.vector.tensor_scalar_add(
                        pre[:, t0 : t0 + CH, jc, :], ps[:], bias[:, jc : jc + 1]
                    )

            def scan(t):
                ps = sps[t % 4]
                nc.tensor.matmul(
                    ps[:], ident[:], pre[:, t, :, :], start=True, stop=False
                )
                if t > 0:
                    for jc in range(4):
                        for kc in range(4):
                            nc.tensor.matmul(
                                ps[:, jc, :],
                                whh[:, kc, jc, :],
                                seq[:, t - 1, kc, :],
                                start=False,
                                stop=(kc == 3),
                            )
                # tanh in two jc-pieces: the jc01 piece retires while the
                # jc23 matmuls still stream, shortening the h(t)->step(t+1)
                # dependency chain vs a single end-of-step activation.
                nc.scalar.activation(seq[:, t, 0:2, :], ps[:, 0:2, :], Tanh)
                nc.scalar.activation(seq[:, t, 2:4, :], ps[:, 2:4, :], Tanh)

            # Full arm-split chunk pipeline. Tile dependencies are
            # emission-trace based, so each arm's producer->consumer order
            # must be self-consistent; the collective's DRAM accesses are
            # tracked on neither side, hence the explicit add_dep edges:
            # even gates the trigger on its send-DMA (via a gpsimd fence in
            # the same arm), odd gates its staging DMA on the collective.
            with tc.If(pid % 2 == 0):
                gemm_L(0)

            for k in range(NCH):
                with tc.If(pid % 2 == 0, label=f"evena{k}"):
                    # Prefetch hint for this core's upcoming far jump over
                    # the odd arm (taken exactly when pid is even).
                    tc.mark_branch_hint_location(
                        f"odda{k}", hint=pid % 2, engines=mybir.ALL_ENGINES
                    )
                    for t in range(k * CH, (k + 1) * CH):
                        scan(t)
                    if k + 1 < NCH:
                        gemm_L(k + 1)
                    snd = nc.sync.dma_start(
                        send_d[k][:], seq[:, k * CH : (k + 1) * CH, :, :]
                    )
                    fence = nc.gpsimd.memset(probe[:], 0)
                    add_dep(fence.ins, snd.ins, True, "trigger after send lands")
                cc = nc.gpsimd.collective_compute(
                    "AllGather",
                    mybir.AluOpType.bypass,
                    replica_groups=PAIRS,
                    ins=[send_d[k][:]],
                    outs=[gath_d[k][:]],
                )
                with tc.If(pid % 2 == 1, label=f"odda{k}"):
                    if k + 1 < NCH:
                        tc.mark_branch_hint_location(
                            f"evena{k + 1}",
                            hint=(pid + 1) % 2,
                            engines=mybir.ALL_ENGINES,
                        )
                    r = nc.gpsimd.dma_start(recv[:, k % NSLOT], gath_d[k][0:128])
                    add_dep(r.ins, cc.ins, True, "recv after AllGather completes")
                    gemm_R(k)
                    for t in range(k * CH, (k + 1) * CH):
                        scan(t)

            with tc.If(pid % 2 == 1, label="fcarm"):
                fps = gps[0]
                for kc in range(4):
                    nc.tensor.matmul(
                        fps[0:C, 0, :],
                        wfc[:, kc, :],
                        seq[:, S - 1, kc, :],
                        start=(kc == 0),
                        stop=(kc == 3),
                    )
                nc.vector.tensor_scalar_add(fco[:], fps[0:C, 0, :], bfc[:])
                nc.sync.dma_start(out_d[:], fco[:])

    nc.compile()
    return nc


def _prep_inputs(inputs):
    bf = ml_dtypes.bfloat16
    f32 = np.float32

    def lhsT_4(w, n_kc):
        # w: [512, n_kc*128] -> [kp, kc, jc, jp]
        return np.ascontiguousarray(
            w.reshape(4, 128, n_kc, 128).transpose(3, 2, 0, 1)
        ).astype(bf)

    wL = lhsT_4(inputs["w_ih0"], 2)
    wR = lhsT_4(inputs["w_ih1"], 4)
    whh0 = lhsT_4(inputs["w_hh0"], 4)
    whh1 = lhsT_4(inputs["w_hh1"], 4)
    wfc = np.ascontiguousarray(
        inputs["w_fc"].reshape(C, 4, 128).transpose(2, 1, 0)
    ).astype(bf)
    b0 = np.ascontiguousarray(
        (inputs["b_ih0"] + inputs["b_hh0"]).reshape(4, 128).T
    ).astype(f32)
    b1 = np.ascontiguousarray(
        (inputs["b_ih1"] + inputs["b_hh1"]).reshape(4, 128).T
    ).astype(f32)
    bfc = inputs["b_fc"].reshape(C, 1).astype(f32)
    ident = np.eye(128, dtype=f32).astype(bf)

    zx = np.zeros((128, 2, S * BL), bf)
    zw2 = np.zeros((128, 2, 4, 128), bf)
    zw4 = np.zeros((128, 4, 4, 128), bf)
    zfc = np.zeros((128, 4, C), bf)
    zbfc = np.zeros((C, 1), f32)

    x = inputs["x"]
    in_maps = []
    for p in range(NCORES // 2):
        xs = x[p * BL : (p + 1) * BL]  # [b, t, i]
        xT = np.ascontiguousarray(
            xs.transpose(2, 1, 0).reshape(2, 128, S * BL).transpose(1, 0, 2)
        ).astype(bf)
        in_maps.append(
            {
                "xT": xT, "wL": wL, "wR": zw4, "whh": whh0, "bias": b0,
                "wfc": zfc, "bfc": zbfc, "ident": ident,
            }
        )
        in_maps.append(
            {
                "xT": zx, "wL": zw2, "wR": wR, "whh": whh1, "bias": b1,
                "wfc": wfc, "bfc": bfc, "ident": ident,
            }
        )
    return in_maps


def kernel(**inputs):
    from concourse import bass_utils

    if "nc" not in _cache:
        _cache["nc"] = _build_nc()
    nc = _cache["nc"]
    in_maps = _prep_inputs(inputs)
    res = bass_utils.run_bass_kernel_spmd(nc, in_maps, core_ids=list(range(NCORES)))
    y = np.concatenate(
        [np.asarray(res.results[2 * p + 1]["out"]).T for p in range(NCORES // 2)],
        axis=0,
    )
    return y.astype(np.float32)


# revision 25
# speedup vs baseline: 1.1350x; 1.1350x over previous
"""2-layer RNN on 8 TRN2 cores — layer-pipelined across core pairs.

Structure: 4 batch groups x 2 pipeline stages. Pair p = (core 2p, core
2p+1) handles batch group p (BL=16 sequences):
  - even core ("stage0"): layer-0 input GEMM (on x) + layer-0 recurrence;
    ships out0 chunks to its partner via pair AllGather.
  - odd core ("stage1"): layer-1 input GEMM (on received out0) + layer-1
    recurrence + FC head.

vs. the data-parallel baseline (each core: both layers, batch 8), each
weight load in the LDWEIGHTS-bound recurrence now serves 16 batch
elements instead of 8, halving per-core scan steps (512 vs 1024).

Per-core divergence uses tc.If(partition_id % 2) branches; the pair
collectives sit outside all control flow, with stage1's send skipped
(its internal send buffer ships garbage that nobody reads).
"""

import numpy as np
import ml_dtypes

B, S, I, H, C = 64, 512, 256, 512, 10
NCORES = 8
BL = B // (NCORES // 2)  # 16 sequences per pair
CH = 32                  # timesteps per shipped chunk
NCH = S // CH
NSLOT = 4                # recv ring slots

_cache = {}


def _build_nc():
    import concourse.bass as bass
    import concourse.bacc as bacc
    import concourse.tile as tile
    from concourse.bass import mybir

    import bass_rust

    f32 = mybir.dt.float32
    bf16 = mybir.dt.bfloat16
    Tanh = mybir.ActivationFunctionType.Tanh
    PAIRS = [[0, 1], [2, 3], [4, 5], [6, 7]]
    add_dep = bass_rust.add_dep_helper

    nc = bacc.Bacc("TRN2", target_bir_lowering=False, debug=False, num_devices=NCORES)

    xT_d = nc.dram_tensor("xT", [128, 2, S * BL], bf16, kind="ExternalInput")
    wL_d = nc.dram_tensor("wL", [128, 2, 4, 128], bf16, kind="ExternalInput")
    wR_d = nc.dram_tensor("wR", [128, 4, 4, 128], bf16, kind="ExternalInput")
    whh_d = nc.dram_tensor("whh", [128, 4, 4, 128], bf16, kind="ExternalInput")
    bias_d = nc.dram_tensor("bias", [128, 4], f32, kind="ExternalInput")
    wfc_d = nc.dram_tensor("wfc", [128, 4, C], bf16, kind="ExternalInput")
    bfc_d = nc.dram_tensor("bfc", [C, 1], f32, kind="ExternalInput")
    id_d = nc.dram_tensor("ident", [128, 128], bf16, kind="ExternalInput")
    out_d = nc.dram_tensor("out", [C, BL], f32, kind="ExternalOutput")

    with tile.TileContext(nc) as tc:
        with tc.tile_pool(name="sb", bufs=1) as sb, tc.tile_pool(
            name="ps", bufs=1, space="PSUM"
        ) as psp, tc.tile_pool(name="dr", bufs=1, space="DRAM") as dr:
            send_d = [
                dr.tile([128, CH, 4, BL], bf16, name=f"send{k}") for k in range(NCH)
            ]
            gath_d = [
                dr.tile([256, CH, 4, BL], bf16, name=f"gath{k}") for k in range(NCH)
            ]
            xT = sb.tile([128, 2, S * BL], bf16)
            pre = sb.tile([128, S, 4, BL], bf16)
            seq = sb.tile([128, S, 4, BL], bf16)
            recv = sb.tile([128, NSLOT, CH, 4, BL], bf16)
            wL = sb.tile([128, 2, 4, 128], bf16)
            wR = sb.tile([128, 4, 4, 128], bf16)
            whh = sb.tile([128, 4, 4, 128], bf16)
            bias = sb.tile([128, 4], f32)
            wfc = sb.tile([128, 4, C], bf16)
            bfc = sb.tile([C, 1], f32)
            ident = sb.tile([128, 128], bf16)
            fco = sb.tile([C, BL], f32)

            pid = nc.partition_id()
            probe = sb.tile([1, 1, 1, 1], bf16)

            # shared prologue: recurrence weights everyone needs
            nc.sync.dma_start(whh[:], whh_d[:])
            nc.sync.dma_start(bias[:], bias_d[:])
            nc.sync.dma_start(ident[:], id_d[:])
            # stage-specific loads
            with tc.If(pid % 2 == 0) as c0:
                nc.sync.dma_start(wL[:], wL_d[:])
                nc.sync.dma_start(xT[:, :, 0 : 2 * CH * BL], xT_d[:, :, 0 : 2 * CH * BL])
                nc.sync.dma_start(
                    xT[:, :, 2 * CH * BL : 8 * CH * BL],
                    xT_d[:, :, 2 * CH * BL : 8 * CH * BL],
                )
                nc.sync.dma_start(xT[:, :, 8 * CH * BL :], xT_d[:, :, 8 * CH * BL :])
            with c0.Else():
                nc.sync.dma_start(wR[:], wR_d[:])
                nc.sync.dma_start(wfc[:], wfc_d[:])
                nc.sync.dma_start(bfc[:], bfc_d[:])

            gps = [psp.tile([128, CH, BL], f32, name=f"gps{i}") for i in range(4)]
            # Recurrence PSUM: separate tiles for the two 8-batch halves
            # (A = batch 0:8, B = 8:16), alternating per 4-step group.
            # Emitting A's and B's 16-MM blocks back-to-back hides each
            # half's tanh(PSUM)->SBUF round trip under the other's MMs.
            sps = [psp.tile([128, 4, 4, BL // 2], f32, name=f"sps{i}") for i in range(4)]

            def gemm_L(k):
                """Stage0 input GEMM for chunk k (from x)."""
                t0 = k * CH
                for jc in range(4):
                    ps = gps[jc]
                    for kc in range(2):
                        nc.tensor.matmul(
                            ps[:],
                            wL[:, kc, jc, :],
                            xT[:, kc, t0 * BL : (t0 + CH) * BL],
                            start=(kc == 0),
                            stop=(kc == 1),
                        )
                    nc.vector.tensor_scalar_add(
                        pre[:, t0 : t0 + CH, jc, :], ps[:], bias[:, jc : jc + 1]
                    )

            def gemm_R(k):
                """Stage1 input GEMM for chunk k (from received out0)."""
                t0 = k * CH
                sl = k % NSLOT
                for jc in range(4):
                    ps = gps[jc]
                    for kc in range(4):
                        nc.tensor.matmul(
                            ps[:],
                            wR[:, kc, jc, :],
                            recv[:, sl, :, kc, :],
                            start=(kc == 0),
                            stop=(kc == 3),
                        )
                    nc.vector.tensor_scalar_add(
                        pre[:, t0 : t0 + CH, jc, :], ps[:], bias[:, jc : jc + 1]
                    )

            def scan(t):
                g = (t // 4) % 2
                sl = t % 4
                hb = BL // 2
                halves = [(sps[g], 0), (sps[2 + g], hb)]
                if sl == 0:
                    for ps, b0 in halves:
                        nc.tensor.matmul(
                            ps[:, 0:4, :, :],
                            ident[:],
                            pre[:, t : t + 4, :, b0 : b0 + hb],
                            start=True,
                            stop=False,
                        )
                if t == 0:
                    for ps, b0 in halves:
                        nc.scalar.activation(
                            seq[:, 0, :, b0 : b0 + hb], ps[:, 0, :, :], Tanh
                        )
                else:
                    for ps, b0 in halves:
                        for jc in range(4):
                            for kc in range(4):
                                nc.tensor.matmul(
                                    ps[:, sl, jc, :],
                                    whh[:, kc, jc, :],
                                    seq[:, t - 1, kc, b0 : b0 + hb],
                                    start=False,
                                    stop=(kc == 3),
                                )
                        nc.scalar.activation(
                            seq[:, t, :, b0 : b0 + hb], ps[:, sl, :, :], Tanh
                        )

            # Full arm-split chunk pipeline. Tile dependencies are
            # emission-trace based, so each arm's producer->consumer order
            # must be self-consistent; the collective's DRAM accesses are
            # tracked on neither side, hence the explicit add_dep edges:
            # even gates the trigger on its send-DMA (via a gpsimd fence in
            # the same arm), odd gates its staging DMA on the collective.
            with tc.If(pid % 2 == 0):
                gemm_L(0)

            for k in range(NCH):
                with tc.If(pid % 2 == 0, label=f"evena{k}"):
                    # Prefetch hint for this core's upcoming far jump over
                    # the odd arm (taken exactly when pid is even).
                    tc.mark_branch_hint_location(
                        f"odda{k}", hint=pid % 2, engines=mybir.ALL_ENGINES
                    )
                    for t in range(k * CH, (k + 1) * CH):
                        scan(t)
                    if k + 1 < NCH:
                        gemm_L(k + 1)
                    snd = nc.sync.dma_start(
                        send_d[k][:], seq[:, k * CH : (k + 1) * CH, :, :]
                    )
                    fence = nc.gpsimd.memset(probe[:], 0)
                    add_dep(fence.ins, snd.ins, True, "trigger after send lands")
                cc = nc.gpsimd.collective_compute(
                    "AllGather",
                    mybir.AluOpType.bypass,
                    replica_groups=PAIRS,
                    ins=[send_d[k][:]],
                    outs=[gath_d[k][:]],
                )
                with tc.If(pid % 2 == 1, label=f"odda{k}"):
                    if k + 1 < NCH:
                        tc.mark_branch_hint_location(
                            f"evena{k + 1}",
                            hint=(pid + 1) % 2,
                            engines=mybir.ALL_ENGINES,
                        )
                    r = nc.gpsimd.dma_start(recv[:, k % NSLOT], gath_d[k][0:128])
                    add_dep(r.ins, cc.ins, True, "recv after AllGather completes")
                    gemm_R(k)
                    for t in range(k * CH, (k + 1) * CH):
                        scan(t)

            with tc.If(pid % 2 == 1):
                fps = gps[0]
                for kc in range(4):
                    nc.tensor.matmul(
                        fps[0:C, 0, :],
                        wfc[:, kc, :],
                        seq[:, S - 1, kc, :],
                        start=(kc == 0),
                        stop=(kc == 3),
                    )
                nc.vector.tensor_scalar_add(fco[:], fps[0:C, 0, :], bfc[:])
                nc.sync.dma_start(out_d[:], fco[:])

    nc.compile()
    return nc


def _prep_inputs(inputs):
    bf = ml_dtypes.bfloat16
    f32 = np.float32

    def lhsT_4(w, n_kc):
        # w: [512, n_kc*128] -> [kp, kc, jc, jp]
        return np.ascontiguousarray(
            w.reshape(4, 128, n_kc, 128).transpose(3, 2, 0, 1)
        ).astype(bf)

    wL = lhsT_4(inputs["w_ih0"], 2)
    wR = lhsT_4(inputs["w_ih1"], 4)
    whh0 = lhsT_4(inputs["w_hh0"], 4)
    whh1 = lhsT_4(inputs["w_hh1"], 4)
    wfc = np.ascontiguousarray(
        inputs["w_fc"].reshape(C, 4, 128).transpose(2, 1, 0)
    ).astype(bf)
    b0 = np.ascontiguousarray(
        (inputs["b_ih0"] + inputs["b_hh0"]).reshape(4, 128).T
    ).astype(f32)
    b1 = np.ascontiguousarray(
        (inputs["b_ih1"] + inputs["b_hh1"]).reshape(4, 128).T
    ).astype(f32)
    bfc = inputs["b_fc"].reshape(C, 1).astype(f32)
    ident = np.eye(128, dtype=f32).astype(bf)

    zx = np.zeros((128, 2, S * BL), bf)
    zw2 = np.zeros((128, 2, 4, 128), bf)
    zw4 = np.zeros((128, 4, 4, 128), bf)
    zfc = np.zeros((128, 4, C), bf)
    zbfc = np.zeros((C, 1), f32)

    x = inputs["x"]
    in_maps = []
    for p in range(NCORES // 2):
        xs = x[p * BL : (p + 1) * BL]  # [b, t, i]
        xT = np.ascontiguousarray(
            xs.transpose(2, 1, 0).reshape(2, 128, S * BL).transpose(1, 0, 2)
        ).astype(bf)
        in_maps.append(
            {
                "xT": xT, "wL": wL, "wR": zw4, "whh": whh0, "bias": b0,
                "wfc": zfc, "bfc": zbfc, "ident": ident,
            }
        )
        in_maps.append(
            {
                "xT": zx, "wL": zw2, "wR": wR, "whh": whh1, "bias": b1,
                "wfc": wfc, "bfc": bfc, "ident": ident,
            }
        )
    return in_maps


def kernel(**inputs):
    from concourse import bass_utils

    if "nc" not in _cache:
        _cache["nc"] = _build_nc()
    nc = _cache["nc"]
    in_maps = _prep_inputs(inputs)
    res = bass_utils.run_bass_kernel_spmd(nc, in_maps, core_ids=list(range(NCORES)))
    y = np.concatenate(
        [np.asarray(res.results[2 * p + 1]["out"]).T for p in range(NCORES // 2)],
        axis=0,
    )
    return y.astype(np.float32)


# revision 26
# speedup vs baseline: 1.3542x; 1.1931x over previous
import numpy as np
import ml_dtypes

B, S, I, H, C = 64, 512, 256, 512, 10
NCORES = 8
BL = B // NCORES
CH = 32
D = 36
NCH = S // CH

_cache = {}


def _build_nc():
    from collections import deque

    import concourse.bass as bass
    import concourse.bacc as bacc
    import concourse.tile as tile
    from concourse.bass import mybir

    f32 = mybir.dt.float32
    bf16 = mybir.dt.bfloat16
    Tanh = mybir.ActivationFunctionType.Tanh

    nc = bacc.Bacc("TRN2", target_bir_lowering=False, debug=False, num_devices=NCORES)

    xT_d = nc.dram_tensor("xT", [128, 2, S * BL], bf16, kind="ExternalInput")
    wih0_d = nc.dram_tensor("wih0", [128, 2, 4, 128], bf16, kind="ExternalInput")
    whh0_d = nc.dram_tensor("whh0", [128, 4, 4, 128], bf16, kind="ExternalInput")
    wih1_d = nc.dram_tensor("wih1", [128, 4, 4, 128], bf16, kind="ExternalInput")
    whh1_d = nc.dram_tensor("whh1", [128, 4, 4, 128], bf16, kind="ExternalInput")
    wfc_d = nc.dram_tensor("wfc", [128, 4, C], bf16, kind="ExternalInput")
    b0_d = nc.dram_tensor("b0", [128, 4], f32, kind="ExternalInput")
    b1_d = nc.dram_tensor("b1", [128, 4], f32, kind="ExternalInput")
    bfc_d = nc.dram_tensor("bfc", [C, 1], f32, kind="ExternalInput")
    id_d = nc.dram_tensor("ident", [128, 128], bf16, kind="ExternalInput")
    out_d = nc.dram_tensor("out", [C, BL], f32, kind="ExternalOutput")

    with tile.TileContext(nc) as tc:
        with tc.tile_pool(name="sb", bufs=1) as sb, tc.tile_pool(
            name="ps", bufs=1, space="PSUM"
        ) as psp:
            xT = sb.tile([128, 2, S * BL], bf16)
            pre0 = sb.tile([128, S, 4, BL], bf16)
            out0 = sb.tile([128, S, 4, BL], bf16)
            pre1 = sb.tile([128, S, 4, BL], bf16)
            wih0 = sb.tile([128, 2, 4, 128], bf16)
            whh0 = sb.tile([128, 4, 4, 128], bf16)
            wih1 = sb.tile([128, 4, 4, 128], bf16)
            whh1 = sb.tile([128, 4, 4, 128], bf16)
            wfc = sb.tile([128, 4, C], bf16)
            b0 = sb.tile([128, 4], f32)
            b1 = sb.tile([128, 4], f32)
            bfc = sb.tile([C, 1], f32)
            ident = sb.tile([128, 128], bf16)
            h1 = sb.tile([128, 2, 4, BL], bf16)
            fco = sb.tile([C, BL], f32)

            nc.sync.dma_start(wih0[:], wih0_d[:])
            nc.sync.dma_start(
                xT[:, :, 0 : CH * BL], xT_d[:, :, 0 : CH * BL]
            )
            nc.sync.dma_start(b0[:], b0_d[:])
            nc.sync.dma_start(ident[:], id_d[:])
            nc.sync.dma_start(
                xT[:, :, CH * BL : 4 * CH * BL], xT_d[:, :, CH * BL : 4 * CH * BL]
            )
            nc.sync.dma_start(whh0[:], whh0_d[:])
            nc.sync.dma_start(
                xT[:, :, 4 * CH * BL :], xT_d[:, :, 4 * CH * BL :]
            )
            for t_sb, t_d in [
                (wih1, wih1_d), (whh1, whh1_d), (b1, b1_d),
                (wfc, wfc_d), (bfc, bfc_d),
            ]:
                nc.sync.dma_start(t_sb[:], t_d[:])

            gps = [psp.tile([128, 64, BL], f32, name=f"gps{i}") for i in range(4)]
            sps = [psp.tile([128, 4, 4, BL], f32, name=f"sps{i}") for i in range(4)]

            def g0_group(k, jc):
                t0 = k * CH
                ps = gps[jc]
                for kc in range(2):
                    nc.tensor.matmul(
                        ps[:, 0:CH, :],
                        wih0[:, kc, jc, :],
                        xT[:, kc, t0 * BL : (t0 + CH) * BL],
                        start=(kc == 0),
                        stop=(kc == 1),
                    )
                nc.vector.tensor_scalar_add(
                    pre0[:, t0 : t0 + CH, jc, :], ps[:, 0:CH, :], b0[:, jc : jc + 1]
                )

            def g1_group(k, jc):
                t0 = k * CH
                ps = gps[jc]
                for kc in range(4):
                    nc.tensor.matmul(
                        ps[:, 0:CH, :],
                        wih1[:, kc, jc, :],
                        out0[:, t0 : t0 + CH, kc, :],
                        start=(kc == 0),
                        stop=(kc == 3),
                    )
                nc.vector.tensor_scalar_add(
                    pre1[:, t0 : t0 + CH, jc, :], ps[:, 0:CH, :], b1[:, jc : jc + 1]
                )

            # (chunk, jc, emit_fn); q1 items gated by min_t
            q0 = deque(
                (k, jc, g0_group) for k in range(1, NCH) for jc in range(4)
            )
            q1 = deque(
                ((k + 1) * CH + 2, k, jc, g1_group) for k in range(NCH) for jc in range(4)
            )

            def pop(t, n=1):
                for _ in range(n):
                    if q1 and q1[0][0] <= t:
                        _, k, jc, fn = q1.popleft()
                        fn(k, jc)
                    elif q0:
                        k, jc, fn = q0.popleft()
                        fn(k, jc)

            def drain_q0(k):
                while q0 and q0[0][0] <= k:
                    kk, jc, fn = q0.popleft()
                    fn(kk, jc)

            def drain_q1(k):
                while q1 and q1[0][1] <= k:
                    _, kk, jc, fn = q1.popleft()
                    fn(kk, jc)

            def scan_step(t, pre, whh, ps, h_out, h_in_fn):
                sl = t % 4
                if sl == 0:
                    # inject pre for this step AND the next 3 (same PSUM bank)
                    nc.tensor.matmul(
                        ps[:, 0:4, :, :], ident[:], pre[:, t : t + 4, :, :],
                        start=True, stop=False,
                    )
                for kc in range(4):
                    for jc in range(4):
                        nc.tensor.matmul(
                            ps[:, sl, jc, :],
                            whh[:, kc, jc, :],
                            h_in_fn(kc),
                            start=False,
                            stop=(kc == 3),
                        )
                nc.scalar.activation(h_out, ps[:, sl, :, :], Tanh)

            def l0_step(t):
                ps = sps[(t // 4) % 2]
                if t == 0:
                    nc.tensor.matmul(
                        ps[:, 0:4, :, :], ident[:], pre0[:, 0:4, :, :],
                        start=True, stop=False,
                    )
                    nc.scalar.activation(out0[:, 0, :, :], ps[:, 0, :, :], Tanh)
                else:
                    scan_step(
                        t, pre0, whh0, ps,
                        out0[:, t, :, :],
                        lambda kc: out0[:, t - 1, kc, :],
                    )

            def l1_step(t):
                ps = sps[2 + (t // 4) % 2]
                if t == 0:
                    nc.tensor.matmul(
                        ps[:, 0:4, :, :], ident[:], pre1[:, 0:4, :, :],
                        start=True, stop=False,
                    )
                    nc.scalar.activation(h1[:, 0, :, :], ps[:, 0, :, :], Tanh)
                else:
                    scan_step(
                        t, pre1, whh1, ps,
                        h1[:, t % 2, :, :],
                        lambda kc: h1[:, (t - 1) % 2, kc, :],
                    )

            for jc in range(4):
                g0_group(0, jc)

            for t in range(S + D):
                if t < S:
                    drain_q0(t // CH)
                    l0_step(t)
                pop(t, 1)
                if t >= D:
                    s = t - D
                    drain_q1(s // CH)
                    l1_step(s)
                    pop(t, 1)

            fps = gps[0]
            for kc in range(4):
                nc.tensor.matmul(
                    fps[0:C, 0, :], wfc[:, kc, :], h1[:, 1, kc, :],
                    start=(kc == 0), stop=(kc == 3),
                )
            nc.vector.tensor_scalar_add(fco[:], fps[0:C, 0, :], bfc[:])
            nc.sync.dma_start(out_d[:], fco[:])

    nc.compile()
    return nc


def _prep_inputs(inputs):
    bf = ml_dtypes.bfloat16
    w_ih0 = inputs["w_ih0"]
    w_hh0 = inputs["w_hh0"]
    w_ih1 = inputs["w_ih1"]
    w_hh1 = inputs["w_hh1"]
    w_fc = inputs["w_fc"]

    def lhsT_4(w, n_kc):
        # w: [512, n_kc*128] -> [kp, kc, jc, jp]
        return np.ascontiguousarray(
            w.reshape(4, 128, n_kc, 128).transpose(3, 2, 0, 1)
        ).astype(bf)

    shared = {
        "wih0": lhsT_4(w_ih0, 2),
        "whh0": lhsT_4(w_hh0, 4),
        "wih1": lhsT_4(w_ih1, 4),
        "whh1": lhsT_4(w_hh1, 4),
        "wfc": np.ascontiguousarray(w_fc.reshape(C, 4, 128).transpose(2, 1, 0)).astype(bf),
        "b0": np.ascontiguousarray(
            (inputs["b_ih0"] + inputs["b_hh0"]).reshape(4, 128).T
        ).astype(np.float32),
        "b1": np.ascontiguousarray(
            (inputs["b_ih1"] + inputs["b_hh1"]).reshape(4, 128).T
        ).astype(np.float32),
        "bfc": inputs["b_fc"].reshape(C, 1).astype(np.float32),
        "ident": np.eye(128, dtype=np.float32).astype(bf),
    }
    x = inputs["x"]
    in_maps = []
    for c in range(NCORES):
        xs = x[c * BL : (c + 1) * BL]  # [b, t, i]
        xT = (
            np.ascontiguousarray(
                xs.transpose(2, 1, 0).reshape(2, 128, S * BL).transpose(1, 0, 2)
            )
        ).astype(bf)
        m = dict(shared)
        m["xT"] = xT
        in_maps.append(m)
    return in_maps


def kernel(**inputs):
    from concourse import bass_utils

    if "nc" not in _cache:
        _cache["nc"] = _build_nc()
    nc = _cache["nc"]
    in_maps = _prep_inputs(inputs)
    res = bass_utils.run_bass_kernel_spmd(nc, in_maps, core_ids=list(range(NCORES)))
    y = np.concatenate(
        [np.asarray(res.results[c]["out"]).T for c in range(NCORES)], axis=0
    )
    return y.astype(np.float32)

